# revision 25
# baseline (speedup 1.0000x reference)
"""Distributed Trainium2 (Bass) kernel for additive (Bahdanau) attention.

Strategy
--------
The reference computes  scores[b,i,j] = sum_d v[d] * tanh(qp[b,i,d] + kp[b,j,d])
which is O(B*Lq*Lk*Dk) transcendental work (134M tanh) if done directly.
We replace tanh with a 5-term sine expansion

    tanh(s) ~= sum_m c_m sin(w_m s)            (fit on s in [-7.2, 7.2])

and use  sin(w(a+b)) = sin(wa+f)cos(wb-f) + cos(wa+f)sin(wb-f)  to turn the
score tensor into fp32r matmuls over 128 "trig feature" rows per frequency:

    scoresT = sum_m Qf_m^T @ Pf_m          [Lk, 256] per core

ACT's Sin is only valid on ~[-3.4, 3.4]; arguments are range-reduced on DVE
with a fused magic-number round (one tensor_scalar per pass).  For the lowest
frequency a phase shift f=pi/4 keeps all arguments inside [-0.5, 0.5] turns
(no reduction needed), which lets the PE start the score matmuls early.

Sharding: 8 cores = 2 batches x 4 query-blocks of 256 rows.  K/V (transposed
on host, bf16) are replicated per batch; no collectives.  attn is produced
transposed ([Lk, 256] bf16 per core) and transposed back on the host.
"""

import sys
import numpy as np

if "/opt/trn_rl_repo" not in sys.path:
    sys.path.insert(0, "/opt/trn_rl_repo")

import ml_dtypes

B, LQ, LK, DM, DK, DV = 2, 1024, 1024, 512, 64, 64
QS = 256          # query rows per core
NCORES = 8
M = 5             # sine terms
LN_EPS = 1e-6
MAGIC = 12582912.0  # 1.5 * 2^23: fused (x+MAGIC)-MAGIC == round(x) on DVE

# tanh(s) ~= sum c_m sin(om_m s), weighted LSQ fit on [-7.2, 7.2]
CS = np.array([1.20989, 0.273641, 0.08294, 0.024901, 0.006387])
OMS = np.array([0.365543, 1.109931, 1.887858, 2.738271, 3.786378])
OMP = OMS / (2 * np.pi)

_CACHE = {}

# F_all layout (free-axis offsets, elements): per-freq trig features
#   [Qf1 | Praw1 | Qf2 Qf3 Qf4 Qf5 | Praw2..5]
QO = [0, 1280, 2304, 3328, 4352]          # Q feature slice offsets (len LK)
PO = [1024, 5376, 5632, 5888, 6144]       # raw P sin slice offsets (len QS)
FTOT = 6400
UTOT = 5120   # round-path scratch: [uQ2..5 (4*LK) | uP2..5 (4*QS)]


def _build():
    """Build (and cache) the Bass graph."""
    if "nc" in _CACHE:
        return _CACHE["nc"]
    import concourse.bacc as bacc
    import concourse.mybir as mybir
    from concourse.tile import TileContext

    f32 = mybir.dt.float32
    f32r = mybir.dt.float32r
    bf16 = mybir.dt.bfloat16
    AF = mybir.ActivationFunctionType
    ALU = mybir.AluOpType

    nc = bacc.Bacc("TRN2", target_bir_lowering=False, debug=False,
                   num_devices=NCORES)

    qT_d = nc.dram_tensor("qT", [DM, QS], bf16, kind="ExternalInput").ap()
    qn_d = nc.dram_tensor("qn", [QS, DM], f32, kind="ExternalInput").ap()
    kT_d = nc.dram_tensor("kT", [DM + 128, LK], bf16, kind="ExternalInput").ap()
    wk2f_d = nc.dram_tensor("wk2f", [M, DM + 128, 128], bf16, kind="ExternalInput").ap()
    vT_d = nc.dram_tensor("vT", [DM, LK], bf16, kind="ExternalInput").ap()
    wq2_d = nc.dram_tensor("wq2", [DM, 128], bf16, kind="ExternalInput").ap()
    wv_d = nc.dram_tensor("wv", [DM, DV], bf16, kind="ExternalInput").ap()
    wfc_d = nc.dram_tensor("wfc", [DV, DM], f32, kind="ExternalInput").ap()
    pco_d = nc.dram_tensor("pco", [128, M], f32, kind="ExternalInput").ap()

    attn_d = nc.dram_tensor("attnT", [LK, QS], bf16, kind="ExternalOutput").ap()
    out_d = nc.dram_tensor("outp", [QS, DM], bf16, kind="ExternalOutput").ap()

    with TileContext(nc) as tc:
        import contextlib
        with contextlib.ExitStack() as ctx:
            const = ctx.enter_context(tc.tile_pool(name="const", bufs=1))
            work = ctx.enter_context(tc.tile_pool(name="work", bufs=1))
            small = ctx.enter_context(tc.tile_pool(name="small", bufs=2))
            ps1 = ctx.enter_context(tc.tile_pool(name="ps1", bufs=1, space="PSUM"))

            # ---------------- inputs -> SBUF (small/critical first) ------
            pco_sb = const.tile([128, M], f32)
            nc.sync.dma_start(out=pco_sb, in_=pco_d)
            wq2_sb = const.tile([128, 4, 128], bf16)
            nc.sync.dma_start(out=wq2_sb, in_=wq2_d.rearrange("(t p) m -> p t m", p=128))
            wv_sb = const.tile([128, 4, DV], bf16)
            nc.sync.dma_start(out=wv_sb, in_=wv_d.rearrange("(t p) m -> p t m", p=128))
            wfc_f = const.tile([64, DM], f32)
            nc.sync.dma_start(out=wfc_f, in_=wfc_d)
            qT_sb = const.tile([128, 4, QS], bf16)
            nc.sync.dma_start(out=qT_sb, in_=qT_d.rearrange("(t p) l -> p t l", p=128))
            kT_sb = const.tile([128, 5, LK], bf16)
            nc.sync.dma_start(out=kT_sb, in_=kT_d.rearrange("(t p) l -> p t l", p=128))
            wk2f_sb = const.tile([128, M, 5, 128], bf16)
            nc.sync.dma_start(out=wk2f_sb,
                              in_=wk2f_d.rearrange("f (t p) m -> p f t m", p=128))
            vT_sb = const.tile([128, 4, LK], bf16)
            nc.sync.dma_start(out=vT_sb, in_=vT_d.rearrange("(t p) l -> p t l", p=128))
            qn_sb = const.tile([128, 2, DM], f32)
            nc.sync.dma_start(out=qn_sb, in_=qn_d.rearrange("(t p) l -> p t l", p=128))

            wfc_r = const.tile([64, DM], f32r)
            nc.vector.tensor_copy(wfc_r[:], wfc_f[:])
            ones_f = const.tile([128, 1], f32)
            nc.vector.memset(ones_f, 1.0)
            ones_r = const.tile([128, 1], f32r)
            nc.vector.tensor_copy(ones_r[:], ones_f[:])
            ones1_f = const.tile([1, 128], f32)
            nc.vector.memset(ones1_f, 1.0)
            ones1_r = const.tile([1, 128], f32r)
            nc.vector.tensor_copy(ones1_r[:], ones1_f[:])
            rsm_sb = const.tile([128, 1], f32)   # bits 0x5f3759df
            nc.vector.memset(rsm_sb, float(np.frombuffer(
                np.uint32(0x5F3759DF).tobytes(), dtype=np.float32)[0]))

            # ------------- projections + trig, pipelined by key halves -----
            # P side first: it only needs qT (small, arrives early)
            qp2_ps = ps1.tile([128, QS], f32, tag="qtb")
            for t in range(4):
                nc.tensor.matmul(qp2_ps[:], wq2_sb[:, t, :], qT_sb[:, t, :],
                                 start=(t == 0), stop=(t == 3))

            rarg = work.tile([128, FTOT], f32)    # pre-sin arguments
            fall = work.tile([128, FTOT], f32r)   # sin outputs (features)
            pfin = work.tile([128, M, QS], f32r)  # coef-scaled P features
            uu = work.tile([128, UTOT], f32)
            nn = work.tile([128, UTOT], f32)
            UP = 4 * LK                           # P-side offset in uu/nn

            qp2_sb = work.tile([128, QS], f32)
            nc.vector.tensor_copy(qp2_sb[:], qp2_ps[:])
            nc.vector.tensor_scalar(out=rarg[:, PO[0]:PO[0] + QS], in0=qp2_sb[:],
                                    scalar1=float(OMP[0]), scalar2=dvs_sb[:, 0:1],
                                    op0=ALU.mult, op1=ALU.add)
            nc.scalar.activation(out=fall[:, PO[0]:PO[0] + QS],
                                 in_=rarg[:, PO[0]:PO[0] + QS],
                                 func=AF.Sin, scale=float(2 * np.pi))
            nc.vector.tensor_scalar(out=pfin[:, 0, :],
                                    in0=fall[:, PO[0]:PO[0] + QS],
                                    scalar1=pco_sb[:, 0:1], scalar2=None,
                                    op0=ALU.mult)
            for i, m in enumerate(range(1, M)):
                nc.vector.tensor_scalar(
                    out=uu[:, UP + i * QS:UP + (i + 1) * QS],
                    in0=qp2_sb[:],
                    scalar1=float(OMP[m]), scalar2=dvP[:],
                    op0=ALU.mult, op1=ALU.add)
            nc.vector.tensor_scalar(out=nn[:, UP:UP + 4 * QS],
                                    in0=uu[:, UP:UP + 4 * QS], scalar1=MAGIC,
                                    scalar2=-MAGIC, op0=ALU.add, op1=ALU.add)
            nc.vector.tensor_tensor(out=rarg[:, PO[1]:PO[1] + 4 * QS],
                                    in0=uu[:, UP:UP + 4 * QS],
                                    in1=nn[:, UP:UP + 4 * QS], op=ALU.subtract)
            nc.scalar.activation(out=fall[:, PO[1]:PO[1] + 4 * QS],
                                 in_=rarg[:, PO[1]:PO[1] + 4 * QS],
                                 func=AF.Sin, scale=float(2 * np.pi))
            for m in range(1, M):
                nc.vector.tensor_scalar(out=pfin[:, m, :],
                                        in0=fall[:, PO[m]:PO[m] + QS],
                                        scalar1=pco_sb[:, m:m + 1], scalar2=None,
                                        op0=ALU.mult)

            # vp with keys on partitions: lhsT = vT chunks (fills PE early)
            vpc_ps = ps1.tile([128, 8, DV], f32, tag="qtb")
            for kc in range(8):
                for t in range(4):
                    nc.tensor.matmul(vpc_ps[:, kc, :],
                                     vT_sb[:, t, kc * 128:(kc + 1) * 128],
                                     wv_sb[:, t, :],
                                     start=(t == 0), stop=(t == 3))
            vp_sb = work.tile([128, 8, DV], f32r)
            nc.vector.tensor_copy(vp_sb[:], vpc_ps[:])

            # Q side, per key-half: per-freq projections emit u directly;
            # delta offsets folded via the kT ones-row (K=1 matmul)
            sc_ps = ps1.tile([128, 2048], f32, tag="sc")   # 8 chunks of 256
            expT = work.tile([128, 8, QS], f32r)
            den_ps = ps1.tile([1, QS], f32, tag="qtb")
            du_ps2 = ps1.tile([64, QS], f32, tag="duv")
            HL = 512
            for kh in range(2):
                ks = kh * HL
                for m in range(M):
                    u_ps = ps1.tile([128, HL], f32, tag="u")
                    for t in range(4):
                        nc.tensor.matmul(u_ps[:], wk2f_sb[:, m, t, :],
                                         kT_sb[:, t, ks:ks + HL],
                                         start=(t == 0), stop=False)
                    nc.tensor.matmul(u_ps[:], wk2f_sb[0:1, m, 4, :],
                                     kT_sb[0:1, 4, ks:ks + HL],
                                     start=False, stop=True)
                    if m == 0:
                        nc.scalar.activation(out=fall[:, QO[0] + ks:QO[0] + ks + HL],
                                             in_=u_ps[:], func=AF.Sin,
                                             scale=float(2 * np.pi))
                    else:
                        i = m - 1
                        nc.vector.tensor_scalar(out=nn[:, i * LK + ks:i * LK + ks + HL],
                                                in0=u_ps[:], scalar1=MAGIC,
                                                scalar2=-MAGIC,
                                                op0=ALU.add, op1=ALU.add)
                        nc.vector.tensor_tensor(out=rarg[:, QO[m] + ks:QO[m] + ks + HL],
                                                in0=nn[:, i * LK + ks:i * LK + ks + HL],
                                                in1=u_ps[:], op=ALU.subtract)
                        nc.scalar.activation(out=fall[:, QO[m] + ks:QO[m] + ks + HL],
                                             in_=rarg[:, QO[m] + ks:QO[m] + ks + HL],
                                             func=AF.Sin, scale=float(-2 * np.pi))
                for kc in range(4 * kh, 4 * kh + 4):
                    for m in range(M):
                        nc.tensor.matmul(
                            sc_ps[:, kc * QS:(kc + 1) * QS],
                            fall[:, QO[m] + kc * 128:QO[m] + (kc + 1) * 128],
                            pfin[:, m, :],
                            start=(m == 0), stop=(m == M - 1))
                for kp_ in range(2 * kh, 2 * kh + 2):
                    nc.scalar.activation(
                        out=expT[:, 2 * kp_:2 * kp_ + 2, :],
                        in_=sc_ps[:, 2 * kp_ * QS:(2 * kp_ + 2) * QS],
                        func=AF.Exp, scale=1.0)
                for kc in range(4 * kh, 4 * kh + 4):
                    nc.tensor.matmul(den_ps[:], ones_r[:], expT[:, kc, :],
                                     start=(kc == 0), stop=(kc == 7))
                    nc.tensor.matmul(du_ps2[:], vp_sb[:, kc, :], expT[:, kc, :],
                                     start=(kc == 0), stop=(kc == 7))

            # ---------------- softmax / av -------------------------------
            rec_f = small.tile([1, QS], f32)
            nc.vector.reciprocal(out=rec_f[:], in_=den_ps[:])
            rec_r = small.tile([1, QS], f32r)
            nc.vector.tensor_copy(rec_r[:], rec_f[:])
            bc_ps = ps1.tile([128, QS], f32, tag="qtb")
            nc.tensor.matmul(bc_ps[:], ones1_r[:], rec_r[:], start=True, stop=True)
            # attn output (normalized, bf16, transposed)
            bc2_sb = work.tile([128, 2, QS], f32)
            nc.vector.tensor_copy(bc2_sb[:, 0, :], bc_ps[:])
            nc.vector.tensor_copy(bc2_sb[:, 1, :], bc_ps[:])
            for kp_ in range(4):
                at_sb = small.tile([128, 2, QS], bf16, tag="at")
                nc.vector.tensor_tensor(out=at_sb[:],
                                        in0=expT[:, 2 * kp_:2 * kp_ + 2, :].bitcast(f32),
                                        in1=bc2_sb[:], op=ALU.mult)
                nc.sync.dma_start(
                    out=attn_d[kp_ * 256:(kp_ + 1) * 256, :].rearrange(
                        "(a p) l -> p a l", p=128),
                    in_=at_sb[:])

            avn = work.tile([64, QS], f32r)
            nc.vector.tensor_tensor(out=avn[:], in0=du_ps2[:],
                                    in1=bc2_sb[0:64, 0, :], op=ALU.mult)

            # ---------------- fc + residual + layernorm -----------------
            for qc in range(2):
                o_ps = ps1.tile([128, DM], f32, tag="u")
                nc.tensor.matmul(o_ps[:], avn[:, qc * 128:(qc + 1) * 128],
                                 wfc_r[:], start=True, stop=True)
                o1 = small.tile([128, DM], f32, tag="o1")
                nc.vector.tensor_tensor(out=o1[:], in0=o_ps[:],
                                        in1=qn_sb[:, qc, :], op=ALU.add)
                st = small.tile([128, 6], f32, tag="st")
                nc.vector.bn_stats(out=st[:], in_=o1[:])
                mv = small.tile([128, 2], f32, tag="mv")
                nc.vector.bn_aggr(out=mv[:], in_=st[:])
                i32 = mybir.dt.int32
                v1 = small.tile([128, 1], f32, tag="v1")
                nc.vector.tensor_scalar(out=v1[:], in0=mv[:, 1:2],
                                        scalar1=LN_EPS, scalar2=None,
                                        op0=ALU.add)
                vh = small.tile([128, 1], f32, tag="vh")
                nc.vector.tensor_scalar(out=vh[:], in0=v1[:], scalar1=-0.5,
                                        scalar2=None, op0=ALU.mult)
                jj = small.tile([128, 1], i32, tag="jj")
                nc.vector.tensor_scalar(out=jj[:], in0=v1[:].bitcast(i32),
                                        scalar1=1, scalar2=None,
                                        op0=ALU.arith_shift_right)
                yb = small.tile([128, 1], i32, tag="yb")
                nc.vector.tensor_tensor(out=yb[:], in0=rsm_sb[:].bitcast(i32),
                                        in1=jj[:], op=ALU.subtract)
                rs = yb[:].bitcast(f32)
                for _ in range(2):
                    t1 = small.tile([128, 1], f32, tag="t1")
                    nc.vector.tensor_tensor(out=t1[:], in0=rs, in1=rs,
                                            op=ALU.mult)
                    t2 = small.tile([128, 1], f32, tag="t2")
                    nc.vector.tensor_scalar(out=t2[:], in0=t1[:],
                                            scalar1=vh[:], scalar2=1.5,
                                            op0=ALU.mult, op1=ALU.add)
                    t3 = small.tile([128, 1], f32, tag="t3")
                    nc.vector.tensor_tensor(out=t3[:], in0=rs, in1=t2[:],
                                            op=ALU.mult)
                    rs = t3[:]
                o4 = small.tile([128, DM], bf16, tag="o4")
                nc.vector.tensor_scalar(out=o4[:], in0=o1[:],
                                        scalar1=mv[:, 0:1], scalar2=rs,
                                        op0=ALU.subtract, op1=ALU.mult)
                nc.sync.dma_start(
                    out=out_d.rearrange("(t p) l -> p t l", p=128)[:, qc, :],
                    in_=o4[:])

    nc.finalize()
    _CACHE["nc"] = nc
    return nc


def _prep_inputs(q, k, v, Wq, Wk, Wv, v_param, Wfc, ln_gamma, ln_beta):
    bf = ml_dtypes.bfloat16
    wq2 = np.ascontiguousarray(np.concatenate([Wq, Wq], axis=1)).astype(bf)
    wk2 = np.ascontiguousarray(np.concatenate([Wk, Wk], axis=1)).astype(bf)
    # per-freq omega-scaled K-projection weights, [M, 512, 128] bf16;
    # delta offsets are folded via an extra all-ones row of kT (row 512)
    wk2f = np.zeros((M, DM + 128, 128), dtype=np.float64)
    wk2_f64 = np.concatenate([Wk, Wk], axis=1).astype(np.float64)
    for m in range(M):
        wk2f[m, :DM, :] = wk2_f64 * OMP[m]
        if m == 0:
            wk2f[m, DM, 0:64] = -0.125   # f1 "-cos" rows
            wk2f[m, DM, 64:128] = 0.125  # f1 sin rows
        else:
            wk2f[m, DM, 0:64] = 0.25     # cos rows
            wk2f[m, DM, 64:128] = 0.0    # sin rows
    wk2f = np.ascontiguousarray(wk2f).astype(bf)
    wv = np.ascontiguousarray(Wv).astype(bf)
    wfc = np.ascontiguousarray(Wfc).astype(np.float32)
    pco = np.zeros((128, M), dtype=np.float32)
    for m in range(M):
        s = -1.0 if m == 0 else 1.0   # f1 sin-rows pair with Q's "-cos" rows
        pco[0:64, m] = (s * CS[m] * v_param).astype(np.float32)
        pco[64:128, m] = (CS[m] * v_param).astype(np.float32)
    dvs = np.zeros((128, 2), dtype=np.float32)
    dvs[0:64, 0] = -0.125   # f1: Q "-cos" rows / P sin rows
    dvs[64:128, 0] = 0.125  # f1: Q sin rows / P cos rows
    gam = np.ascontiguousarray(ln_gamma.reshape(1, DM)).astype(np.float32)
    bet = np.ascontiguousarray(ln_beta.reshape(1, DM)).astype(np.float32)
    eye = np.eye(64, dtype=np.float32)

    kT = []
    for b in range(B):
        kTb = np.zeros((DM + 128, LK), dtype=np.float32)
        kTb[:DM] = k[b].T
        kTb[DM] = 1.0
        kT.append(np.ascontiguousarray(kTb).astype(bf))
    vT = [np.ascontiguousarray(v[b].T).astype(bf) for b in range(B)]

    in_maps = []
    for core in range(NCORES):
        b, q0 = core // 4, QS * (core % 4)
        qs = q[b, q0:q0 + QS]
        in_maps.append({
            "qT": np.ascontiguousarray(qs.T).astype(bf),
            "wk2f": wk2f,
            "qn": np.ascontiguousarray(qs).astype(np.float32),
            "kT": kT[b],
            "vT": vT[b],
            "wq2": wq2, "wv": wv, "wfc": wfc,
            "pco": pco,
        })
    return in_maps


def kernel(q, k, v, Wq, Wk, Wv, v_param, Wfc, ln_gamma, ln_beta,
           _trace=False):
    q = np.asarray(q); k = np.asarray(k); v = np.asarray(v)
    Wq = np.asarray(Wq); Wk = np.asarray(Wk); Wv = np.asarray(Wv)
    v_param = np.asarray(v_param); Wfc = np.asarray(Wfc)
    ln_gamma = np.asarray(ln_gamma); ln_beta = np.asarray(ln_beta)

    from concourse.bass_utils import run_bass_kernel_spmd

    nc = _build()
    in_maps = _prep_inputs(q, k, v, Wq, Wk, Wv, v_param, Wfc, ln_gamma, ln_beta)
    res = run_bass_kernel_spmd(nc, in_maps, core_ids=list(range(NCORES)),
                               trace=_trace)

    out = np.zeros((B, LQ, DM), dtype=np.float32)
    attn = np.zeros((B, LQ, LK), dtype=np.float32)
    for core in range(NCORES):
        b, q0 = core // 4, QS * (core % 4)
        out[b, q0:q0 + QS] = res.results[core]["outp"].astype(np.float32)
        attn[b, q0:q0 + QS] = res.results[core]["attnT"].astype(np.float32).T
    # ln_gamma/ln_beta are identity in this model config; guard anyway
    if not (np.all(ln_gamma == 1.0) and np.all(ln_beta == 0.0)):
        out = (out * ln_gamma.astype(np.float32)
               + ln_beta.astype(np.float32)).astype(np.float32)
    if _trace:
        kernel.last_exec_time_ns = res.exec_time_ns
    return out, attn


# revision 26
# speedup vs baseline: 1.3610x; 1.3610x over previous
"""Distributed Trainium2 (Bass) kernel for additive (Bahdanau) attention.

Strategy
--------
The reference computes  scores[b,i,j] = sum_d v[d] * tanh(qp[b,i,d] + kp[b,j,d])
which is O(B*Lq*Lk*Dk) transcendental work (134M tanh) if done directly.
We replace tanh with a 5-term sine expansion

    tanh(s) ~= sum_m c_m sin(w_m s)            (fit on s in [-7.2, 7.2])

and use  sin(w(a+b)) = sin(wa+f)cos(wb-f) + cos(wa+f)sin(wb-f)  to turn the
score tensor into fp32r matmuls over 128 "trig feature" rows per frequency:

    scoresT = sum_m Qf_m^T @ Pf_m          [Lk, 256] per core

ACT's Sin is only valid on ~[-3.4, 3.4]; arguments are range-reduced on DVE
with a fused magic-number round (one tensor_scalar per pass).  For the lowest
frequency a phase shift f=pi/4 keeps all arguments inside [-0.5, 0.5] turns
(no reduction needed), which lets the PE start the score matmuls early.

Sharding: 8 cores = 2 batches x 4 query-blocks of 256 rows.  K/V (transposed
on host, bf16) are replicated per batch; no collectives.  attn is produced
transposed ([Lk, 256] bf16 per core) and transposed back on the host.
"""

import sys
import numpy as np

if "/opt/trn_rl_repo" not in sys.path:
    sys.path.insert(0, "/opt/trn_rl_repo")

import ml_dtypes

B, LQ, LK, DM, DK, DV = 2, 1024, 1024, 512, 64, 64
QS = 256          # query rows per core
NCORES = 8
M = 5             # sine terms
LN_EPS = 1e-6
MAGIC = 12582912.0  # 1.5 * 2^23: fused (x+MAGIC)-MAGIC == round(x) on DVE

# tanh(s) ~= sum c_m sin(om_m s), weighted LSQ fit on [-7.2, 7.2]
CS = np.array([1.20989, 0.273641, 0.08294, 0.024901, 0.006387])
OMS = np.array([0.365543, 1.109931, 1.887858, 2.738271, 3.786378])
OMP = OMS / (2 * np.pi)

_CACHE = {}

# F_all layout (free-axis offsets, elements): per-freq trig features
#   [Qf1 | Praw1 | Qf2 Qf3 Qf4 Qf5 | Praw2..5]
QO = [0, 1280, 2304, 3328, 4352]          # Q feature slice offsets (len LK)
PO = [1024, 5376, 5632, 5888, 6144]       # raw P sin slice offsets (len QS)
FTOT = 6400
UTOT = 5120   # round-path scratch: [uQ2..5 (4*LK) | uP2..5 (4*QS)]


def _build():
    """Build (and cache) the Bass graph."""
    if "nc" in _CACHE:
        return _CACHE["nc"]
    import concourse.bacc as bacc
    import concourse.mybir as mybir
    from concourse.tile import TileContext

    f32 = mybir.dt.float32
    f32r = mybir.dt.float32r
    bf16 = mybir.dt.bfloat16
    AF = mybir.ActivationFunctionType
    ALU = mybir.AluOpType

    nc = bacc.Bacc("TRN2", target_bir_lowering=False, debug=False,
                   num_devices=NCORES)

    qT_d = nc.dram_tensor("qT", [DM, QS], bf16, kind="ExternalInput").ap()
    qn_d = nc.dram_tensor("qn", [QS, DM], f32, kind="ExternalInput").ap()
    kT_d = nc.dram_tensor("kT", [DM + 128, LK], bf16, kind="ExternalInput").ap()
    wk2f_d = nc.dram_tensor("wk2f", [M, DM + 128, 128], bf16, kind="ExternalInput").ap()
    vT_d = nc.dram_tensor("vT", [DM, LK], bf16, kind="ExternalInput").ap()
    wq2_d = nc.dram_tensor("wq2", [DM, 128], bf16, kind="ExternalInput").ap()
    wv_d = nc.dram_tensor("wv", [DM, DV], bf16, kind="ExternalInput").ap()
    wfc_d = nc.dram_tensor("wfc", [DV, DM], f32, kind="ExternalInput").ap()
    pco_d = nc.dram_tensor("pco", [128, M], f32, kind="ExternalInput").ap()

    attn_d = nc.dram_tensor("attnT", [LK, QS], bf16, kind="ExternalOutput").ap()
    out_d = nc.dram_tensor("outp", [QS, DM], bf16, kind="ExternalOutput").ap()

    with TileContext(nc) as tc:
        import contextlib
        with contextlib.ExitStack() as ctx:
            const = ctx.enter_context(tc.tile_pool(name="const", bufs=1))
            work = ctx.enter_context(tc.tile_pool(name="work", bufs=1))
            small = ctx.enter_context(tc.tile_pool(name="small", bufs=2))
            ps1 = ctx.enter_context(tc.tile_pool(name="ps1", bufs=1, space="PSUM"))
            psu = ctx.enter_context(tc.tile_pool(name="psu", bufs=2, space="PSUM"))

            # ---------------- inputs -> SBUF (small/critical first) ------
            pco_sb = const.tile([128, M], f32)
            nc.sync.dma_start(out=pco_sb, in_=pco_d)
            wq2_sb = const.tile([128, 4, 128], bf16)
            nc.sync.dma_start(out=wq2_sb, in_=wq2_d.rearrange("(t p) m -> p t m", p=128))
            wv_sb = const.tile([128, 4, DV], bf16)
            nc.sync.dma_start(out=wv_sb, in_=wv_d.rearrange("(t p) m -> p t m", p=128))
            wfc_f = const.tile([64, DM], f32)
            nc.sync.dma_start(out=wfc_f, in_=wfc_d)
            qT_sb = const.tile([128, 4, QS], bf16)
            nc.sync.dma_start(out=qT_sb, in_=qT_d.rearrange("(t p) l -> p t l", p=128))
            kT_sb = const.tile([128, 5, LK], bf16)
            nc.sync.dma_start(out=kT_sb, in_=kT_d.rearrange("(t p) l -> p t l", p=128))
            wk2f_sb = const.tile([128, M, 5, 128], bf16)
            nc.sync.dma_start(out=wk2f_sb,
                              in_=wk2f_d.rearrange("f (t p) m -> p f t m", p=128))
            vT_sb = const.tile([128, 4, LK], bf16)
            nc.sync.dma_start(out=vT_sb, in_=vT_d.rearrange("(t p) l -> p t l", p=128))
            qn_sb = const.tile([128, 2, DM], f32)
            nc.sync.dma_start(out=qn_sb, in_=qn_d.rearrange("(t p) l -> p t l", p=128))

            wfc_r = const.tile([64, DM], f32r)
            nc.vector.tensor_copy(wfc_r[:], wfc_f[:])
            ones_f = const.tile([128, 1], f32)
            nc.vector.memset(ones_f, 1.0)
            ones_r = const.tile([128, 1], f32r)
            nc.vector.tensor_copy(ones_r[:], ones_f[:])
            ones1_f = const.tile([1, 128], f32)
            nc.vector.memset(ones1_f, 1.0)
            ones1_r = const.tile([1, 128], f32r)
            nc.vector.tensor_copy(ones1_r[:], ones1_f[:])
            rsm_sb = const.tile([128, 1], f32)   # bits 0x5f3759df
            nc.vector.memset(rsm_sb, float(np.frombuffer(
                np.uint32(0x5F3759DF).tobytes(), dtype=np.float32)[0]))

            # ------------- projections + trig, pipelined by key halves -----
            # P side first: it only needs qT (small, arrives early)
            qp2_ps = ps1.tile([128, QS], f32, tag="qtb")
            for t in range(4):
                nc.tensor.matmul(qp2_ps[:], wq2_sb[:, t, :], qT_sb[:, t, :],
                                 start=(t == 0), stop=(t == 3))

            rarg = work.tile([128, FTOT], f32)    # pre-sin arguments
            fall = work.tile([128, FTOT], f32r)   # sin outputs (features)
            pfin = work.tile([128, M, QS], f32r)  # coef-scaled P features
            uu = work.tile([128, UTOT], f32)
            nn = work.tile([128, UTOT], f32)
            UP = 4 * LK                           # P-side offset in uu/nn

            qp2_sb = work.tile([128, QS], f32)
            nc.vector.tensor_copy(qp2_sb[:], qp2_ps[:])
            nc.vector.tensor_scalar(out=rarg[:, PO[0]:PO[0] + QS], in0=qp2_sb[:],
                                    scalar1=float(OMP[0]), scalar2=dvs_sb[:, 0:1],
                                    op0=ALU.mult, op1=ALU.add)
            nc.scalar.activation(out=fall[:, PO[0]:PO[0] + QS],
                                 in_=rarg[:, PO[0]:PO[0] + QS],
                                 func=AF.Sin, scale=float(2 * np.pi))
            nc.vector.tensor_scalar(out=pfin[:, 0, :],
                                    in0=fall[:, PO[0]:PO[0] + QS],
                                    scalar1=pco_sb[:, 0:1], scalar2=None,
                                    op0=ALU.mult)
            for i, m in enumerate(range(1, M)):
                nc.vector.tensor_scalar(
                    out=uu[:, UP + i * QS:UP + (i + 1) * QS],
                    in0=qp2_sb[:],
                    scalar1=float(OMP[m]), scalar2=dvP[:],
                    op0=ALU.mult, op1=ALU.add)
            nc.vector.tensor_scalar(out=nn[:, UP:UP + 4 * QS],
                                    in0=uu[:, UP:UP + 4 * QS], scalar1=MAGIC,
                                    scalar2=-MAGIC, op0=ALU.add, op1=ALU.add)
            nc.vector.tensor_tensor(out=rarg[:, PO[1]:PO[1] + 4 * QS],
                                    in0=uu[:, UP:UP + 4 * QS],
                                    in1=nn[:, UP:UP + 4 * QS], op=ALU.subtract)
            nc.scalar.activation(out=fall[:, PO[1]:PO[1] + 4 * QS],
                                 in_=rarg[:, PO[1]:PO[1] + 4 * QS],
                                 func=AF.Sin, scale=float(2 * np.pi))
            for m in range(1, M):
                nc.vector.tensor_scalar(out=pfin[:, m, :],
                                        in0=fall[:, PO[m]:PO[m] + QS],
                                        scalar1=pco_sb[:, m:m + 1], scalar2=None,
                                        op0=ALU.mult)

            # vp with keys on partitions: lhsT = vT chunks (fills PE early)
            vpc_ps = ps1.tile([128, 8, DV], f32, tag="qtb")
            for kc in range(8):
                for t in range(4):
                    nc.tensor.matmul(vpc_ps[:, kc, :],
                                     vT_sb[:, t, kc * 128:(kc + 1) * 128],
                                     wv_sb[:, t, :],
                                     start=(t == 0), stop=(t == 3))
            vp_sb = work.tile([128, 8, DV], f32r)
            nc.vector.tensor_copy(vp_sb[:], vpc_ps[:])

            # Q side, per key-half: per-freq projections emit u directly;
            # delta offsets folded via the kT ones-row (K=1 matmul)
            sc_ps = ps1.tile([128, 2048], f32, tag="sc")   # 8 chunks of 256
            expT = work.tile([128, 8, QS], f32r)
            den_ps = ps1.tile([1, QS], f32, tag="qtb")
            du_ps2 = ps1.tile([64, QS], f32, tag="duv")
            HL = 512
            for kh in range(2):
                ks = kh * HL
                for m in range(M):
                    u_ps = psu.tile([128, HL], f32, tag="u")
                    for t in range(4):
                        nc.tensor.matmul(u_ps[:], wk2f_sb[:, m, t, :],
                                         kT_sb[:, t, ks:ks + HL],
                                         start=(t == 0), stop=False)
                    nc.tensor.matmul(u_ps[:], wk2f_sb[0:1, m, 4, :],
                                     kT_sb[0:1, 4, ks:ks + HL],
                                     start=False, stop=True)
                    if m == 0:
                        nc.scalar.activation(out=fall[:, QO[0] + ks:QO[0] + ks + HL],
                                             in_=u_ps[:], func=AF.Sin,
                                             scale=float(2 * np.pi))
                    else:
                        i = m - 1
                        nc.vector.tensor_scalar(out=nn[:, i * LK + ks:i * LK + ks + HL],
                                                in0=u_ps[:], scalar1=MAGIC,
                                                scalar2=-MAGIC,
                                                op0=ALU.add, op1=ALU.add)
                        nc.vector.tensor_tensor(out=rarg[:, QO[m] + ks:QO[m] + ks + HL],
                                                in0=nn[:, i * LK + ks:i * LK + ks + HL],
                                                in1=u_ps[:], op=ALU.subtract)
                        nc.scalar.activation(out=fall[:, QO[m] + ks:QO[m] + ks + HL],
                                             in_=rarg[:, QO[m] + ks:QO[m] + ks + HL],
                                             func=AF.Sin, scale=float(-2 * np.pi))
                for kc in range(4 * kh, 4 * kh + 4):
                    for m in range(M):
                        nc.tensor.matmul(
                            sc_ps[:, kc * QS:(kc + 1) * QS],
                            fall[:, QO[m] + kc * 128:QO[m] + (kc + 1) * 128],
                            pfin[:, m, :],
                            start=(m == 0), stop=(m == M - 1))
                for kp_ in range(2 * kh, 2 * kh + 2):
                    nc.scalar.activation(
                        out=expT[:, 2 * kp_:2 * kp_ + 2, :],
                        in_=sc_ps[:, 2 * kp_ * QS:(2 * kp_ + 2) * QS],
                        func=AF.Exp, scale=1.0)
                for kc in range(4 * kh, 4 * kh + 4):
                    nc.tensor.matmul(den_ps[:], ones_r[:], expT[:, kc, :],
                                     start=(kc == 0), stop=(kc == 7))
                    nc.tensor.matmul(du_ps2[:], vp_sb[:, kc, :], expT[:, kc, :],
                                     start=(kc == 0), stop=(kc == 7))

            # ---------------- softmax / av -------------------------------
            rec_f = small.tile([1, QS], f32)
            nc.vector.reciprocal(out=rec_f[:], in_=den_ps[:])
            rec_r = small.tile([1, QS], f32r)
            nc.vector.tensor_copy(rec_r[:], rec_f[:])
            bc_ps = ps1.tile([128, QS], f32, tag="qtb")
            nc.tensor.matmul(bc_ps[:], ones1_r[:], rec_r[:], start=True, stop=True)
            # attn output (normalized, bf16, transposed)
            bc2_sb = work.tile([128, 2, QS], f32)
            nc.vector.tensor_copy(bc2_sb[:, 0, :], bc_ps[:])
            nc.vector.tensor_copy(bc2_sb[:, 1, :], bc_ps[:])
            for kp_ in range(4):
                at_sb = small.tile([128, 2, QS], bf16, tag="at")
                nc.vector.tensor_tensor(out=at_sb[:],
                                        in0=expT[:, 2 * kp_:2 * kp_ + 2, :].bitcast(f32),
                                        in1=bc2_sb[:], op=ALU.mult)
                nc.sync.dma_start(
                    out=attn_d[kp_ * 256:(kp_ + 1) * 256, :].rearrange(
                        "(a p) l -> p a l", p=128),
                    in_=at_sb[:])

            avn = work.tile([64, QS], f32r)
            nc.vector.tensor_tensor(out=avn[:], in0=du_ps2[:],
                                    in1=bc2_sb[0:64, 0, :], op=ALU.mult)

            # ---------------- fc + residual + layernorm -----------------
            for qc in range(2):
                o_ps = psu.tile([128, DM], f32, tag="u")
                nc.tensor.matmul(o_ps[:], avn[:, qc * 128:(qc + 1) * 128],
                                 wfc_r[:], start=True, stop=True)
                o1 = small.tile([128, DM], f32, tag="o1")
                nc.vector.tensor_tensor(out=o1[:], in0=o_ps[:],
                                        in1=qn_sb[:, qc, :], op=ALU.add)
                st = small.tile([128, 6], f32, tag="st")
                nc.vector.bn_stats(out=st[:], in_=o1[:])
                mv = small.tile([128, 2], f32, tag="mv")
                nc.vector.bn_aggr(out=mv[:], in_=st[:])
                i32 = mybir.dt.int32
                v1 = small.tile([128, 1], f32, tag="v1")
                nc.vector.tensor_scalar(out=v1[:], in0=mv[:, 1:2],
                                        scalar1=LN_EPS, scalar2=None,
                                        op0=ALU.add)
                vh = small.tile([128, 1], f32, tag="vh")
                nc.vector.tensor_scalar(out=vh[:], in0=v1[:], scalar1=-0.5,
                                        scalar2=None, op0=ALU.mult)
                jj = small.tile([128, 1], i32, tag="jj")
                nc.vector.tensor_scalar(out=jj[:], in0=v1[:].bitcast(i32),
                                        scalar1=1, scalar2=None,
                                        op0=ALU.arith_shift_right)
                yb = small.tile([128, 1], i32, tag="yb")
                nc.vector.tensor_tensor(out=yb[:], in0=rsm_sb[:].bitcast(i32),
                                        in1=jj[:], op=ALU.subtract)
                rs = yb[:].bitcast(f32)
                for _ in range(2):
                    t1 = small.tile([128, 1], f32, tag="t1")
                    nc.vector.tensor_tensor(out=t1[:], in0=rs, in1=rs,
                                            op=ALU.mult)
                    t2 = small.tile([128, 1], f32, tag="t2")
                    nc.vector.tensor_scalar(out=t2[:], in0=t1[:],
                                            scalar1=vh[:], scalar2=1.5,
                                            op0=ALU.mult, op1=ALU.add)
                    t3 = small.tile([128, 1], f32, tag="t3")
                    nc.vector.tensor_tensor(out=t3[:], in0=rs, in1=t2[:],
                                            op=ALU.mult)
                    rs = t3[:]
                o4 = small.tile([128, DM], bf16, tag="o4")
                nc.vector.tensor_scalar(out=o4[:], in0=o1[:],
                                        scalar1=mv[:, 0:1], scalar2=rs,
                                        op0=ALU.subtract, op1=ALU.mult)
                nc.sync.dma_start(
                    out=out_d.rearrange("(t p) l -> p t l", p=128)[:, qc, :],
                    in_=o4[:])

    nc.finalize()
    _CACHE["nc"] = nc
    return nc


def _prep_inputs(q, k, v, Wq, Wk, Wv, v_param, Wfc, ln_gamma, ln_beta):
    bf = ml_dtypes.bfloat16
    wq2 = np.ascontiguousarray(np.concatenate([Wq, Wq], axis=1)).astype(bf)
    wk2 = np.ascontiguousarray(np.concatenate([Wk, Wk], axis=1)).astype(bf)
    # per-freq omega-scaled K-projection weights, [M, 512, 128] bf16;
    # delta offsets are folded via an extra all-ones row of kT (row 512)
    wk2f = np.zeros((M, DM + 128, 128), dtype=np.float64)
    wk2_f64 = np.concatenate([Wk, Wk], axis=1).astype(np.float64)
    for m in range(M):
        wk2f[m, :DM, :] = wk2_f64 * OMP[m]
        if m == 0:
            wk2f[m, DM, 0:64] = -0.125   # f1 "-cos" rows
            wk2f[m, DM, 64:128] = 0.125  # f1 sin rows
        else:
            wk2f[m, DM, 0:64] = 0.25     # cos rows
            wk2f[m, DM, 64:128] = 0.0    # sin rows
    wk2f = np.ascontiguousarray(wk2f).astype(bf)
    wv = np.ascontiguousarray(Wv).astype(bf)
    wfc = np.ascontiguousarray(Wfc).astype(np.float32)
    pco = np.zeros((128, M), dtype=np.float32)
    for m in range(M):
        s = -1.0 if m == 0 else 1.0   # f1 sin-rows pair with Q's "-cos" rows
        pco[0:64, m] = (s * CS[m] * v_param).astype(np.float32)
        pco[64:128, m] = (CS[m] * v_param).astype(np.float32)
    dvs = np.zeros((128, 2), dtype=np.float32)
    dvs[0:64, 0] = -0.125   # f1: Q "-cos" rows / P sin rows
    dvs[64:128, 0] = 0.125  # f1: Q sin rows / P cos rows
    gam = np.ascontiguousarray(ln_gamma.reshape(1, DM)).astype(np.float32)
    bet = np.ascontiguousarray(ln_beta.reshape(1, DM)).astype(np.float32)
    eye = np.eye(64, dtype=np.float32)

    kT = []
    for b in range(B):
        kTb = np.zeros((DM + 128, LK), dtype=np.float32)
        kTb[:DM] = k[b].T
        kTb[DM] = 1.0
        kT.append(np.ascontiguousarray(kTb).astype(bf))
    vT = [np.ascontiguousarray(v[b].T).astype(bf) for b in range(B)]

    in_maps = []
    for core in range(NCORES):
        b, q0 = core // 4, QS * (core % 4)
        qs = q[b, q0:q0 + QS]
        in_maps.append({
            "qT": np.ascontiguousarray(qs.T).astype(bf),
            "wk2f": wk2f,
            "qn": np.ascontiguousarray(qs).astype(np.float32),
            "kT": kT[b],
            "vT": vT[b],
            "wq2": wq2, "wv": wv, "wfc": wfc,
            "pco": pco,
        })
    return in_maps


def kernel(q, k, v, Wq, Wk, Wv, v_param, Wfc, ln_gamma, ln_beta,
           _trace=False):
    q = np.asarray(q); k = np.asarray(k); v = np.asarray(v)
    Wq = np.asarray(Wq); Wk = np.asarray(Wk); Wv = np.asarray(Wv)
    v_param = np.asarray(v_param); Wfc = np.asarray(Wfc)
    ln_gamma = np.asarray(ln_gamma); ln_beta = np.asarray(ln_beta)

    from concourse.bass_utils import run_bass_kernel_spmd

    nc = _build()
    in_maps = _prep_inputs(q, k, v, Wq, Wk, Wv, v_param, Wfc, ln_gamma, ln_beta)
    res = run_bass_kernel_spmd(nc, in_maps, core_ids=list(range(NCORES)),
                               trace=_trace)

    out = np.zeros((B, LQ, DM), dtype=np.float32)
    attn = np.zeros((B, LQ, LK), dtype=np.float32)
    for core in range(NCORES):
        b, q0 = core // 4, QS * (core % 4)
        out[b, q0:q0 + QS] = res.results[core]["outp"].astype(np.float32)
        attn[b, q0:q0 + QS] = res.results[core]["attnT"].astype(np.float32).T
    # ln_gamma/ln_beta are identity in this model config; guard anyway
    if not (np.all(ln_gamma == 1.0) and np.all(ln_beta == 0.0)):
        out = (out * ln_gamma.astype(np.float32)
               + ln_beta.astype(np.float32)).astype(np.float32)
    if _trace:
        kernel.last_exec_time_ns = res.exec_time_ns
    return out, attn


# revision 27
# speedup vs baseline: 1.3978x; 1.0270x over previous
"""Distributed Trainium2 (Bass) kernel for additive (Bahdanau) attention.

Strategy
--------
The reference computes  scores[b,i,j] = sum_d v[d] * tanh(qp[b,i,d] + kp[b,j,d])
which is O(B*Lq*Lk*Dk) transcendental work (134M tanh) if done directly.
We replace tanh with a 5-term sine expansion

    tanh(s) ~= sum_m c_m sin(w_m s)            (fit on s in [-7.2, 7.2])

and use  sin(w(a+b)) = sin(wa+f)cos(wb-f) + cos(wa+f)sin(wb-f)  to turn the
score tensor into fp32r matmuls over 128 "trig feature" rows per frequency:

    scoresT = sum_m Qf_m^T @ Pf_m          [Lk, 256] per core

ACT's Sin is only valid on ~[-3.4, 3.4]; arguments are range-reduced on DVE
with a fused magic-number round (one tensor_scalar per pass).  For the lowest
frequency a phase shift f=pi/4 keeps all arguments inside [-0.5, 0.5] turns
(no reduction needed), which lets the PE start the score matmuls early.

Sharding: 8 cores = 2 batches x 4 query-blocks of 256 rows.  K/V (transposed
on host, bf16) are replicated per batch; no collectives.  attn is produced
transposed ([Lk, 256] bf16 per core) and transposed back on the host.
"""

import sys
import numpy as np

if "/opt/trn_rl_repo" not in sys.path:
    sys.path.insert(0, "/opt/trn_rl_repo")

import ml_dtypes

B, LQ, LK, DM, DK, DV = 2, 1024, 1024, 512, 64, 64
QS = 256          # query rows per core
NCORES = 8
M = 5             # sine terms
LN_EPS = 1e-6
MAGIC = 12582912.0  # 1.5 * 2^23: fused (x+MAGIC)-MAGIC == round(x) on DVE

# tanh(s) ~= sum c_m sin(om_m s), weighted LSQ fit on [-7.2, 7.2]
CS = np.array([1.20989, 0.273641, 0.08294, 0.024901, 0.006387])
OMS = np.array([0.365543, 1.109931, 1.887858, 2.738271, 3.786378])
OMP = OMS / (2 * np.pi)

_CACHE = {}

# F_all layout (free-axis offsets, elements): per-freq trig features
#   [Qf1 | Praw1 | Qf2 Qf3 Qf4 Qf5 | Praw2..5]
QO = [0, 1280, 2304, 3328, 4352]          # Q feature slice offsets (len LK)
PO = [1024, 5376, 5632, 5888, 6144]       # raw P sin slice offsets (len QS)
FTOT = 6400
UTOT = 5120   # round-path scratch: [uQ2..5 (4*LK) | uP2..5 (4*QS)]


def _build():
    """Build (and cache) the Bass graph."""
    if "nc" in _CACHE:
        return _CACHE["nc"]
    import concourse.bacc as bacc
    import concourse.mybir as mybir
    from concourse.tile import TileContext

    f32 = mybir.dt.float32
    f32r = mybir.dt.float32r
    bf16 = mybir.dt.bfloat16
    AF = mybir.ActivationFunctionType
    ALU = mybir.AluOpType

    nc = bacc.Bacc("TRN2", target_bir_lowering=False, debug=False,
                   num_devices=NCORES)

    qT_d = nc.dram_tensor("qT", [DM, QS], bf16, kind="ExternalInput").ap()
    qn_d = nc.dram_tensor("qn", [QS, DM], f32, kind="ExternalInput").ap()
    kT_d = nc.dram_tensor("kT", [DM, LK], bf16, kind="ExternalInput").ap()
    wk2f_d = nc.dram_tensor("wk2f", [M, DM + 128, 128], bf16, kind="ExternalInput").ap()
    vT_d = nc.dram_tensor("vT", [DM, LK], bf16, kind="ExternalInput").ap()
    wq2_d = nc.dram_tensor("wq2", [DM, 128], bf16, kind="ExternalInput").ap()
    wv_d = nc.dram_tensor("wv", [DM, DV], bf16, kind="ExternalInput").ap()
    wfc_d = nc.dram_tensor("wfc", [DV, DM], f32, kind="ExternalInput").ap()
    pco_d = nc.dram_tensor("pco", [128, M], f32, kind="ExternalInput").ap()

    attn_d = nc.dram_tensor("attnT", [LK, QS], bf16, kind="ExternalOutput").ap()
    out_d = nc.dram_tensor("outp", [QS, DM], bf16, kind="ExternalOutput").ap()

    with TileContext(nc) as tc:
        import contextlib
        with contextlib.ExitStack() as ctx:
            const = ctx.enter_context(tc.tile_pool(name="const", bufs=1))
            work = ctx.enter_context(tc.tile_pool(name="work", bufs=1))
            small = ctx.enter_context(tc.tile_pool(name="small", bufs=2))
            ps1 = ctx.enter_context(tc.tile_pool(name="ps1", bufs=1, space="PSUM"))
            psu = ctx.enter_context(tc.tile_pool(name="psu", bufs=2, space="PSUM"))

            # ---------------- inputs -> SBUF (small/critical first) ------
            pco_sb = const.tile([128, M], f32)
            nc.sync.dma_start(out=pco_sb, in_=pco_d)
            wq2_sb = const.tile([128, 4, 128], bf16)
            nc.sync.dma_start(out=wq2_sb, in_=wq2_d.rearrange("(t p) m -> p t m", p=128))
            wv_sb = const.tile([128, 4, DV], bf16)
            nc.sync.dma_start(out=wv_sb, in_=wv_d.rearrange("(t p) m -> p t m", p=128))
            wfc_f = const.tile([64, DM], f32)
            nc.sync.dma_start(out=wfc_f, in_=wfc_d)
            qT_sb = const.tile([128, 4, QS], bf16)
            nc.sync.dma_start(out=qT_sb, in_=qT_d.rearrange("(t p) l -> p t l", p=128))
            kT_sb = const.tile([128, 5, LK], bf16)
            nc.sync.dma_start(out=kT_sb[:, 0:4, :],
                              in_=kT_d.rearrange("(t p) l -> p t l", p=128))
            nc.gpsimd.memset(kT_sb[0:1, 4, :], 1.0)
            wk2f_sb = const.tile([128, M, 5, 128], bf16)
            nc.sync.dma_start(out=wk2f_sb,
                              in_=wk2f_d.rearrange("f (t p) m -> p f t m", p=128))
            vT_sb = const.tile([128, 4, LK], bf16)
            nc.sync.dma_start(out=vT_sb, in_=vT_d.rearrange("(t p) l -> p t l", p=128))
            qn_sb = const.tile([128, 2, DM], f32)
            nc.sync.dma_start(out=qn_sb, in_=qn_d.rearrange("(t p) l -> p t l", p=128))

            wfc_r = const.tile([64, DM], f32r)
            nc.vector.tensor_copy(wfc_r[:], wfc_f[:])
            ones_f = const.tile([128, 1], f32)
            nc.vector.memset(ones_f, 1.0)
            ones_r = const.tile([128, 1], bf16)
            nc.vector.tensor_copy(ones_r[:], ones_f[:])
            ones1_f = const.tile([1, 128], f32)
            nc.vector.memset(ones1_f, 1.0)
            ones1_r = const.tile([1, 128], f32r)
            nc.vector.tensor_copy(ones1_r[:], ones1_f[:])
            rsm_sb = const.tile([128, 1], f32)   # bits 0x5f3759df
            nc.vector.memset(rsm_sb, float(np.frombuffer(
                np.uint32(0x5F3759DF).tobytes(), dtype=np.float32)[0]))

            # ------------- projections + trig, pipelined by key halves -----
            # P side first: it only needs qT (small, arrives early)
            qp2_ps = ps1.tile([128, QS], f32, tag="qtb")
            for t in range(4):
                nc.tensor.matmul(qp2_ps[:], wq2_sb[:, t, :], qT_sb[:, t, :],
                                 start=(t == 0), stop=(t == 3))

            rarg = work.tile([128, FTOT], f32)    # pre-sin arguments
            fall = work.tile([128, FTOT], bf16)   # sin outputs (features)
            pfin = work.tile([128, M, QS], bf16)  # coef-scaled P features
            uu = work.tile([128, UTOT], f32)
            nn = work.tile([128, UTOT], f32)
            UP = 4 * LK                           # P-side offset in uu/nn

            qp2_sb = work.tile([128, QS], f32)
            nc.vector.tensor_copy(qp2_sb[:], qp2_ps[:])
            nc.vector.tensor_scalar(out=rarg[:, PO[0]:PO[0] + QS], in0=qp2_sb[:],
                                    scalar1=float(OMP[0]), scalar2=dvs_sb[:, 0:1],
                                    op0=ALU.mult, op1=ALU.add)
            nc.scalar.activation(out=fall[:, PO[0]:PO[0] + QS],
                                 in_=rarg[:, PO[0]:PO[0] + QS],
                                 func=AF.Sin, scale=float(2 * np.pi))
            nc.vector.tensor_scalar(out=pfin[:, 0, :],
                                    in0=fall[:, PO[0]:PO[0] + QS],
                                    scalar1=pco_sb[:, 0:1], scalar2=None,
                                    op0=ALU.mult)
            for i, m in enumerate(range(1, M)):
                nc.vector.tensor_scalar(
                    out=uu[:, UP + i * QS:UP + (i + 1) * QS],
                    in0=qp2_sb[:],
                    scalar1=float(OMP[m]), scalar2=dvP[:],
                    op0=ALU.mult, op1=ALU.add)
            nc.vector.tensor_scalar(out=nn[:, UP:UP + 4 * QS],
                                    in0=uu[:, UP:UP + 4 * QS], scalar1=MAGIC,
                                    scalar2=-MAGIC, op0=ALU.add, op1=ALU.add)
            nc.vector.tensor_tensor(out=rarg[:, PO[1]:PO[1] + 4 * QS],
                                    in0=uu[:, UP:UP + 4 * QS],
                                    in1=nn[:, UP:UP + 4 * QS], op=ALU.subtract)
            nc.scalar.activation(out=fall[:, PO[1]:PO[1] + 4 * QS],
                                 in_=rarg[:, PO[1]:PO[1] + 4 * QS],
                                 func=AF.Sin, scale=float(2 * np.pi))
            for m in range(1, M):
                nc.vector.tensor_scalar(out=pfin[:, m, :],
                                        in0=fall[:, PO[m]:PO[m] + QS],
                                        scalar1=pco_sb[:, m:m + 1], scalar2=None,
                                        op0=ALU.mult)

            # vp with keys on partitions: lhsT = vT chunks (fills PE early)
            vpc_ps = ps1.tile([128, 8, DV], f32, tag="qtb")
            for kc in range(8):
                for t in range(4):
                    nc.tensor.matmul(vpc_ps[:, kc, :],
                                     vT_sb[:, t, kc * 128:(kc + 1) * 128],
                                     wv_sb[:, t, :],
                                     start=(t == 0), stop=(t == 3))
            vp_sb = work.tile([128, 8, DV], bf16)
            nc.vector.tensor_copy(vp_sb[:], vpc_ps[:])

            # Q side, per key-half: per-freq projections emit u directly;
            # delta offsets folded via the kT ones-row (K=1 matmul)
            sc_ps = ps1.tile([128, 2048], f32, tag="sc")   # 8 chunks of 256
            expT = work.tile([128, 8, QS], bf16)
            den_ps = ps1.tile([1, QS], f32, tag="qtb")
            du_ps2 = ps1.tile([64, QS], f32, tag="duv")
            HL = 512
            for kh in range(2):
                ks = kh * HL
                for m in range(M):
                    u_ps = psu.tile([128, HL], f32, tag="u")
                    for t in range(4):
                        nc.tensor.matmul(u_ps[:], wk2f_sb[:, m, t, :],
                                         kT_sb[:, t, ks:ks + HL],
                                         start=(t == 0), stop=False)
                    nc.tensor.matmul(u_ps[:], wk2f_sb[0:1, m, 4, :],
                                     kT_sb[0:1, 4, ks:ks + HL],
                                     start=False, stop=True)
                    if m == 0:
                        nc.scalar.activation(out=fall[:, QO[0] + ks:QO[0] + ks + HL],
                                             in_=u_ps[:], func=AF.Sin,
                                             scale=float(2 * np.pi))
                    else:
                        i = m - 1
                        nc.vector.tensor_scalar(out=nn[:, i * LK + ks:i * LK + ks + HL],
                                                in0=u_ps[:], scalar1=MAGIC,
                                                scalar2=-MAGIC,
                                                op0=ALU.add, op1=ALU.add)
                        nc.vector.tensor_tensor(out=rarg[:, QO[m] + ks:QO[m] + ks + HL],
                                                in0=nn[:, i * LK + ks:i * LK + ks + HL],
                                                in1=u_ps[:], op=ALU.subtract)
                        nc.scalar.activation(out=fall[:, QO[m] + ks:QO[m] + ks + HL],
                                             in_=rarg[:, QO[m] + ks:QO[m] + ks + HL],
                                             func=AF.Sin, scale=float(-2 * np.pi))
                for kc in range(4 * kh, 4 * kh + 4):
                    for m in range(M):
                        nc.tensor.matmul(
                            sc_ps[:, kc * QS:(kc + 1) * QS],
                            fall[:, QO[m] + kc * 128:QO[m] + (kc + 1) * 128],
                            pfin[:, m, :],
                            start=(m == 0), stop=(m == M - 1))
                for kp_ in range(2 * kh, 2 * kh + 2):
                    nc.scalar.activation(
                        out=expT[:, 2 * kp_:2 * kp_ + 2, :],
                        in_=sc_ps[:, 2 * kp_ * QS:(2 * kp_ + 2) * QS],
                        func=AF.Exp, scale=1.0)
                for kc in range(4 * kh, 4 * kh + 4):
                    nc.tensor.matmul(den_ps[:], ones_r[:], expT[:, kc, :],
                                     start=(kc == 0), stop=(kc == 7))
                    nc.tensor.matmul(du_ps2[:], vp_sb[:, kc, :], expT[:, kc, :],
                                     start=(kc == 0), stop=(kc == 7))

            # ---------------- softmax / av -------------------------------
            rec_f = small.tile([1, QS], f32)
            nc.vector.reciprocal(out=rec_f[:], in_=den_ps[:])
            rec_r = small.tile([1, QS], f32r)
            nc.vector.tensor_copy(rec_r[:], rec_f[:])
            bc_ps = ps1.tile([128, QS], f32, tag="qtb")
            nc.tensor.matmul(bc_ps[:], ones1_r[:], rec_r[:], start=True, stop=True)
            # attn output (normalized, bf16, transposed)
            bc2_sb = work.tile([128, 2, QS], bf16)
            nc.vector.tensor_copy(bc2_sb[:, 0, :], bc_ps[:])
            nc.vector.tensor_copy(bc2_sb[:, 1, :], bc_ps[:])
            bc64_sb = work.tile([64, QS], f32)
            nc.vector.tensor_copy(bc64_sb[:], bc_ps[0:64, :])
            for kp_ in range(4):
                at_sb = small.tile([128, 2, QS], bf16, tag="at")
                nc.vector.tensor_tensor(out=at_sb[:],
                                        in0=expT[:, 2 * kp_:2 * kp_ + 2, :],
                                        in1=bc2_sb[:], op=ALU.mult)
                nc.sync.dma_start(
                    out=attn_d[kp_ * 256:(kp_ + 1) * 256, :].rearrange(
                        "(a p) l -> p a l", p=128),
                    in_=at_sb[:])

            avn = work.tile([64, QS], f32r)
            nc.vector.tensor_tensor(out=avn[:], in0=du_ps2[:],
                                    in1=bc64_sb[:], op=ALU.mult)

            # ---------------- fc + residual + layernorm -----------------
            for qc in range(2):
                o_ps = psu.tile([128, DM], f32, tag="u")
                nc.tensor.matmul(o_ps[:], avn[:, qc * 128:(qc + 1) * 128],
                                 wfc_r[:], start=True, stop=True)
                o1 = small.tile([128, DM], f32, tag="o1")
                nc.vector.tensor_tensor(out=o1[:], in0=o_ps[:],
                                        in1=qn_sb[:, qc, :], op=ALU.add)
                st = small.tile([128, 6], f32, tag="st")
                nc.vector.bn_stats(out=st[:], in_=o1[:])
                mv = small.tile([128, 2], f32, tag="mv")
                nc.vector.bn_aggr(out=mv[:], in_=st[:])
                i32 = mybir.dt.int32
                v1 = small.tile([128, 1], f32, tag="v1")
                nc.vector.tensor_scalar(out=v1[:], in0=mv[:, 1:2],
                                        scalar1=LN_EPS, scalar2=None,
                                        op0=ALU.add)
                vh = small.tile([128, 1], f32, tag="vh")
                nc.vector.tensor_scalar(out=vh[:], in0=v1[:], scalar1=-0.5,
                                        scalar2=None, op0=ALU.mult)
                jj = small.tile([128, 1], i32, tag="jj")
                nc.vector.tensor_scalar(out=jj[:], in0=v1[:].bitcast(i32),
                                        scalar1=1, scalar2=None,
                                        op0=ALU.arith_shift_right)
                yb = small.tile([128, 1], i32, tag="yb")
                nc.vector.tensor_tensor(out=yb[:], in0=rsm_sb[:].bitcast(i32),
                                        in1=jj[:], op=ALU.subtract)
                rs = yb[:].bitcast(f32)
                for _ in range(2):
                    t1 = small.tile([128, 1], f32, tag="t1")
                    nc.vector.tensor_tensor(out=t1[:], in0=rs, in1=rs,
                                            op=ALU.mult)
                    t2 = small.tile([128, 1], f32, tag="t2")
                    nc.vector.tensor_scalar(out=t2[:], in0=t1[:],
                                            scalar1=vh[:], scalar2=1.5,
                                            op0=ALU.mult, op1=ALU.add)
                    t3 = small.tile([128, 1], f32, tag="t3")
                    nc.vector.tensor_tensor(out=t3[:], in0=rs, in1=t2[:],
                                            op=ALU.mult)
                    rs = t3[:]
                o4 = small.tile([128, DM], bf16, tag="o4")
                nc.vector.tensor_scalar(out=o4[:], in0=o1[:],
                                        scalar1=mv[:, 0:1], scalar2=rs,
                                        op0=ALU.subtract, op1=ALU.mult)
                nc.sync.dma_start(
                    out=out_d.rearrange("(t p) l -> p t l", p=128)[:, qc, :],
                    in_=o4[:])

    nc.finalize()
    _CACHE["nc"] = nc
    return nc


def _prep_inputs(q, k, v, Wq, Wk, Wv, v_param, Wfc, ln_gamma, ln_beta):
    bf = ml_dtypes.bfloat16
    wq2 = np.ascontiguousarray(np.concatenate([Wq, Wq], axis=1)).astype(bf)
    wk2 = np.ascontiguousarray(np.concatenate([Wk, Wk], axis=1)).astype(bf)
    # per-freq omega-scaled K-projection weights, [M, 512, 128] bf16;
    # delta offsets are folded via an extra all-ones row of kT (row 512)
    wk2f = np.zeros((M, DM + 128, 128), dtype=np.float64)
    wk2_f64 = np.concatenate([Wk, Wk], axis=1).astype(np.float64)
    for m in range(M):
        wk2f[m, :DM, :] = wk2_f64 * OMP[m]
        if m == 0:
            wk2f[m, DM, 0:64] = -0.125   # f1 "-cos" rows
            wk2f[m, DM, 64:128] = 0.125  # f1 sin rows
        else:
            wk2f[m, DM, 0:64] = 0.25     # cos rows
            wk2f[m, DM, 64:128] = 0.0    # sin rows
    wk2f = np.ascontiguousarray(wk2f).astype(bf)
    wv = np.ascontiguousarray(Wv).astype(bf)
    wfc = np.ascontiguousarray(Wfc).astype(np.float32)
    pco = np.zeros((128, M), dtype=np.float32)
    for m in range(M):
        s = -1.0 if m == 0 else 1.0   # f1 sin-rows pair with Q's "-cos" rows
        pco[0:64, m] = (s * CS[m] * v_param).astype(np.float32)
        pco[64:128, m] = (CS[m] * v_param).astype(np.float32)
    dvs = np.zeros((128, 2), dtype=np.float32)
    dvs[0:64, 0] = -0.125   # f1: Q "-cos" rows / P sin rows
    dvs[64:128, 0] = 0.125  # f1: Q sin rows / P cos rows
    gam = np.ascontiguousarray(ln_gamma.reshape(1, DM)).astype(np.float32)
    bet = np.ascontiguousarray(ln_beta.reshape(1, DM)).astype(np.float32)
    eye = np.eye(64, dtype=np.float32)

    kT = [np.ascontiguousarray(k[b].T).astype(bf) for b in range(B)]
    vT = [np.ascontiguousarray(v[b].T).astype(bf) for b in range(B)]

    in_maps = []
    for core in range(NCORES):
        b, q0 = core // 4, QS * (core % 4)
        qs = q[b, q0:q0 + QS]
        in_maps.append({
            "qT": np.ascontiguousarray(qs.T).astype(bf),
            "wk2f": wk2f,
            "qn": np.ascontiguousarray(qs).astype(np.float32),
            "kT": kT[b],
            "vT": vT[b],
            "wq2": wq2, "wv": wv, "wfc": wfc,
            "pco": pco,
        })
    return in_maps


def kernel(q, k, v, Wq, Wk, Wv, v_param, Wfc, ln_gamma, ln_beta,
           _trace=False):
    q = np.asarray(q); k = np.asarray(k); v = np.asarray(v)
    Wq = np.asarray(Wq); Wk = np.asarray(Wk); Wv = np.asarray(Wv)
    v_param = np.asarray(v_param); Wfc = np.asarray(Wfc)
    ln_gamma = np.asarray(ln_gamma); ln_beta = np.asarray(ln_beta)

    from concourse.bass_utils import run_bass_kernel_spmd

    nc = _build()
    in_maps = _prep_inputs(q, k, v, Wq, Wk, Wv, v_param, Wfc, ln_gamma, ln_beta)
    res = run_bass_kernel_spmd(nc, in_maps, core_ids=list(range(NCORES)),
                               trace=_trace)

    out = np.zeros((B, LQ, DM), dtype=np.float32)
    attn = np.zeros((B, LQ, LK), dtype=np.float32)
    for core in range(NCORES):
        b, q0 = core // 4, QS * (core % 4)
        out[b, q0:q0 + QS] = res.results[core]["outp"].astype(np.float32)
        attn[b, q0:q0 + QS] = res.results[core]["attnT"].astype(np.float32).T
    # ln_gamma/ln_beta are identity in this model config; guard anyway
    if not (np.all(ln_gamma == 1.0) and np.all(ln_beta == 0.0)):
        out = (out * ln_gamma.astype(np.float32)
               + ln_beta.astype(np.float32)).astype(np.float32)
    if _trace:
        kernel.last_exec_time_ns = res.exec_time_ns
    return out, attn
